# revision 6
# baseline (speedup 1.0000x reference)
"""Trainium2 Bass kernel for nn_CausalStructureLearner.

adjacency[b,i,j] = sigmoid(sum_h W2[h]*relu(ai[b,i,h]+aj[b,j,h]+b1[h]) + b2)
                   * (1-eye)
structural = broadcast(structure_params)

Math rewrite vs the straightforward version:
  W2[h]*relu(x) = sign(W2[h]) * relu(|W2[h]|*x), so |W2[h]| is folded into
  the W1 columns (and b1) on the host and h is permuted so all
  positive-sign h's come first. The PE-side reduction over h then uses only
  two stationary matrices (+I / -I fp16), replacing the 2MB block-diagonal
  w2i constant. b_enc is folded into b1 (b1eff), the diagonal mask and the
  fp16->fp32 cast are applied on the host, and adjacency is written fp16.

Per core (batch sharded 4/core across 8 cores), fp16 hot path:
  prep (PE): cfbT -> nfT [h_enc,i]; ajb=W1b'.T@nfT + b1eff [h,j] (DRAM
  round trip so rows can be partition-broadcast); ai = nfT.T@W1a' [i,h].
  main: four per-batch PSUM accumulation chains over h=0..63, interleaved
  round-robin and skewed one step apart (chain b handles h = g-b):
    DMA:  broadcast ajb rows h..h+7 across 128 partitions (fp16)
    DVE (chains 0-2 + tail of 3) / ACT (chain 3, h<ACT_H):
          hid[:,t,:] = relu(bcast + ai[:,t,h] per-partition bias)
    PE:   ps_adj[b] +/-= hid   (+I/-I stationary, [128,512] fp32 acc)
  post (as each chain ends): ACT sigmoid(+b2) PSUM -> fp16 SBUF -> DMA out.

_split_waits(): this container's neuronxcc walrus accepts only one
sync-wait per ISA instruction; extras are hoisted into standalone
EventSemaphore instructions on the same engine.
"""

import os
import sys

sys.path.insert(0, "/opt/trn_rl_repo")

import numpy as np

import bass_rust
import concourse.bass as bass
import concourse.tile as tile
from concourse import mybir
from concourse.bass_utils import run_bass_kernel_spmd

B, N, F_, H = 32, 256, 256, 64
NCORES = 8
BPC = B // NCORES  # batches per core
P = 128  # partitions
HB = 8  # h-rows broadcast per DMA chunk
NOCT = H // HB  # broadcast chunks per batch
ACT_H = 60  # chain 3 h's below this go to ACT, rest to DVE

_CACHE = {}
LAST_RESULT = None  # test harness can read exec_time_ns from here


def _bcast_rows(ap, nparts):
    """AP that reads a [k, n] slice broadcast to [nparts, k, n] partitions."""
    return bass.AP(
        tensor=ap.tensor,
        offset=ap.offset,
        ap=[[0, nparts]] + [list(d) for d in ap.ap],
    )


def _split_waits(nc, keep=1):
    """Walrus (neuronxcc codegen) only supports one sync-wait per ISA
    instruction; Tile emits several. Hoist extras into standalone
    EventSemaphore instructions on the same engine, just before."""
    n = 0
    for f in nc.m.functions:
        for blk in f.blocks:
            new = []
            for ins in blk.instructions:
                si = ins.sync_info
                if si is not None and len(si.on_wait) > keep:
                    extra, kept = si.on_wait[:-keep], si.on_wait[-keep:]
                    for w in extra:
                        ev = mybir.InstEventSemaphore(name=f"I-wsplit-{n}")
                        n += 1
                        ev.engine = ins.engine
                        ev.sync_info = bass_rust.SyncInfo(on_wait=[w], on_update=[])
                        new.append(ev)
                    ins.sync_info = bass_rust.SyncInfo(
                        on_wait=kept, on_update=si.on_update
                    )
                new.append(ins)
            blk.instructions = new
    return n


def _build(hp):
    """hp = number of h's whose (permuted) W2 sign is positive."""
    nc = bass.Bass()
    f32 = mybir.dt.float32
    hf = mybir.dt.float16

    cfb = nc.dram_tensor("cfb", [BPC, F_, N], hf, kind="ExternalInput")
    # consts pack: [:, 0:128]=I, [:, 128:256]=-I, [:, 256:384]=wenc (2,64)
    cw = nc.dram_tensor("cw", [P, 3 * P], hf, kind="ExternalInput")
    w1 = nc.dram_tensor("w1", [H, 2 * H], hf, kind="ExternalInput")  # [w1a'|w1b']
    bb = nc.dram_tensor("bb", [P, 2], f32, kind="ExternalInput")  # b2 | b1eff
    adj = nc.dram_tensor("adj", [BPC, N, N], hf, kind="ExternalOutput")
    ajb_d = nc.dram_tensor("ajb_d", [BPC, H, N], hf)  # broadcast scratch

    AF = mybir.ActivationFunctionType
    OP = mybir.AluOpType

    with tile.TileContext(nc) as tc:
        with (
            tc.tile_pool(name="consts", bufs=1) as consts,
            tc.tile_pool(name="cfbp", bufs=4) as cfbp,
            tc.tile_pool(name="small", bufs=4) as small,
            tc.tile_pool(name="in0p", bufs=10) as in0p,
            tc.tile_pool(name="hidp", bufs=8) as hidp,
            tc.tile_pool(name="hidap", bufs=4) as hidap,
            tc.tile_pool(name="outp", bufs=4) as outp,
            tc.tile_pool(name="gatep", bufs=2) as gatep,
            tc.tile_pool(name="pprep", bufs=3, space="PSUM") as pprep,
            tc.tile_pool(name="padj", bufs=1, space="PSUM") as padj,
        ):
            # ---- first input + consts (all tiny) ----
            cfbT = [None] * BPC
            cfbT[0] = cfbp.tile([P, 2, N], hf, tag="cfbT", name="cfbT0")
            nc.sync.dma_start(
                out=cfbT[0], in_=cfb[0].rearrange("(k p) i -> p k i", p=P)
            )
            cw_sb = consts.tile([P, 3 * P], hf)
            nc.sync.dma_start(out=cw_sb, in_=cw[:])
            w1_sb = consts.tile([H, 2 * H], hf)
            nc.sync.dma_start(out=w1_sb, in_=w1[:])
            bb_sb = consts.tile([P, 2], f32)
            nc.sync.dma_start(out=bb_sb, in_=bb[:])
            ident = cw_sb[:, 0:P]
            nident = cw_sb[:, P : 2 * P]
            b2_sb = bb_sb[:, 0:1]
            b1_sb = bb_sb[0:H, 1:2]

            ai_d = [None] * BPC
            ai_a = [None] * BPC
            ps_adj = [None] * BPC

            def prep(b):
                # nfT [h_enc, i] = W_enc.T @ cfb.T  (b_enc folded into b1eff)
                ps_nf = pprep.tile([H, N], f32, tag="pp")
                for k in range(2):
                    nc.tensor.matmul(
                        ps_nf,
                        cw_sb[:, 2 * P + k * H : 2 * P + (k + 1) * H],
                        cfbT[b][:, k, :],
                        start=(k == 0),
                        stop=(k == 1),
                    )
                nf_sb = small.tile([H, N], hf, tag="nf")
                nc.vector.tensor_copy(nf_sb, ps_nf)

                # ajb [h, j] = W1b'.T @ nfT + b1eff -> DRAM for broadcast
                ps_aj = pprep.tile([H, N], f32, tag="pp")
                nc.tensor.matmul(ps_aj, w1_sb[:, H:], nf_sb, start=True, stop=True)
                ajb_sb = small.tile([H, N], hf, tag="ajb")
                nc.scalar.add(ajb_sb, ps_aj, b1_sb)
                nc.sync.dma_start(out=ajb_d[b], in_=ajb_sb)

                # ai [i, h] = (nfT slice).T @ W1a'   (one copy per consumer
                # engine so cross-engine waits stay within walrus limits)
                ai_d[b] = small.tile([P, 2, H], f32, tag="ai_d", name=f"ai_d{b}")
                if b == BPC - 1:
                    ai_a[b] = small.tile([P, 2, H], f32, tag="ai_a", name=f"ai_a{b}")
                for t in range(2):
                    ps_ai = pprep.tile([P, H], f32, tag="pp")
                    nc.tensor.matmul(
                        ps_ai,
                        nf_sb[:, t * P : (t + 1) * P],
                        w1_sb[:, 0:H],
                        start=True,
                        stop=True,
                    )
                    nc.vector.tensor_copy(ai_d[b][:, t, :], ps_ai)
                    if b == BPC - 1:
                        nc.scalar.copy(ai_a[b][:, t, :], ps_ai)
                ps_adj[b] = padj.tile([P, 2 * N], f32, tag=f"ps_adj{b}", name=f"ps_adj{b}")

            in0s = {}
            in0_cur = {}

            def bcast(b, o):
                in0 = in0p.tile([P, HB, N], hf, tag="in0")
                nc.sync.dma_start(
                    out=in0, in_=_bcast_rows(ajb_d[b, o * HB : (o + 1) * HB, :], P)
                )
                in0s[b] = in0

            def main_step(g):
                for b in range(BPC):
                    h = g - b
                    if not (0 <= h < H):
                        continue
                    if h % HB == 0:
                        in0_cur[b] = in0s[b]  # the prefetched octet h//HB
                    elif h % HB == HB // 2 and h // HB + 1 < NOCT:
                        bcast(b, h // HB + 1)  # mid-octet: ~3.4us of lead
                    use_act = b == BPC - 1 and h < ACT_H
                    if use_act:
                        hid = hidap.tile([P, 2, N], hf, tag="hid_a")
                    else:
                        hid = hidp.tile([P, 2, N], hf, tag="hid")
                    for t in range(2):
                        if use_act:
                            nc.scalar.activation(
                                hid[:, t, :], in0_cur[b][:, h % HB, :], AF.Relu,
                                bias=ai_a[b][:, t, h : h + 1], scale=1.0,
                            )
                        else:
                            nc.vector.tensor_scalar(
                                hid[:, t, :], in0_cur[b][:, h % HB, :],
                                ai_d[b][:, t, h : h + 1], 0.0,
                                OP.add, OP.max,
                            )
                    nc.tensor.matmul(
                        ps_adj[b],
                        ident if h < hp else nident,
                        hid,
                        start=(h == 0),
                        stop=(h == H - 1),
                    )

                if g >= H - 1:
                    b = g - (H - 1)
                    sig = outp.tile([P, 2, N], hf, tag="sig")
                    nc.scalar.activation(
                        sig, ps_adj[b], AF.Sigmoid, bias=b2_sb, scale=1.0
                    )
                    nc.sync.dma_start(
                        out=adj[b].rearrange("(t p) j -> p t j", p=P), in_=sig
                    )

            # ---- pipelined preamble: prep(b) interleaved with the first
            # main-loop steps; later cfb loads issue from the ACT engine's
            # DMA queue so they don't delay chain 0's critical path on SP ----
            for b in range(1, BPC):
                cfbT[b] = cfbp.tile([P, 2, N], hf, tag="cfbT", name=f"cfbT{b}")
                nc.scalar.dma_start(
                    out=cfbT[b], in_=cfb[b].rearrange("(k p) i -> p k i", p=P)
                )
            prep(0)
            bcast(0, 0)
            for b in range(1, BPC):
                prep(b)
                bcast(b, 0)
                main_step(b - 1)

            # ---- main: 4 interleaved accumulation chains, h-outer ----
            for g in range(BPC - 1, H + BPC - 1):
                main_step(g)

    _split_waits(nc)
    return nc


def kernel(causal_factors_batch, W_enc, b_enc, W1, b1, W2, b2, structure_params):
    global LAST_RESULT
    cfb = np.asarray(causal_factors_batch, dtype=np.float32)
    W_enc = np.asarray(W_enc, dtype=np.float32)
    b_enc = np.asarray(b_enc, dtype=np.float32)
    W1 = np.asarray(W1, dtype=np.float32)
    b1 = np.asarray(b1, dtype=np.float32).reshape(-1)
    W2 = np.asarray(W2, dtype=np.float32).reshape(-1)
    b2 = np.asarray(b2, dtype=np.float32).reshape(-1)
    structure_params = np.asarray(structure_params, dtype=np.float32)

    hf = np.float16

    # fold |W2| into the W1 halves / b1, fold b_enc in, sort positives first
    signs = np.where(W2 >= 0, 1.0, -1.0).astype(np.float32)
    order = np.argsort(-signs, kind="stable")
    hp = int((signs > 0).sum())
    absw2 = np.abs(W2)[order]
    w1a = (W1[:H][:, order] * absw2[None, :]).astype(hf)
    w1b = (W1[H:][:, order] * absw2[None, :]).astype(hf)
    b1eff = (b1 + b_enc @ W1[:H] + b_enc @ W1[H:])[order] * absw2

    if ("nc", hp) not in _CACHE:
        _CACHE["nc", hp] = _build(hp)
    nc = _CACHE["nc", hp]

    eye = np.eye(P, dtype=np.float32)
    cw_np = np.concatenate(
        [eye, -eye, W_enc.reshape(2, P, H).transpose(1, 0, 2).reshape(P, 2 * H)],
        axis=1,
    ).astype(hf)
    w1_np = np.concatenate([w1a, w1b], axis=1)
    bb_np = np.zeros((P, 2), dtype=np.float32)
    bb_np[:, 0] = float(b2[0])
    bb_np[:H, 1] = b1eff

    shared = {"cw": cw_np, "w1": w1_np, "bb": bb_np}
    in_maps = []
    for c in range(NCORES):
        m = dict(shared)
        m["cfb"] = np.ascontiguousarray(
            cfb[c * BPC : (c + 1) * BPC].transpose(0, 2, 1)
        ).astype(hf)
        in_maps.append(m)

    trace = bool(os.environ.get("BASS_TRACE"))
    res = run_bass_kernel_spmd(nc, in_maps, list(range(NCORES)), trace=trace)
    LAST_RESULT = res

    adjacency = np.concatenate(
        [res.results[c]["adj"] for c in range(NCORES)], axis=0
    ).astype(np.float32)
    idx = np.arange(N)
    adjacency[:, idx, idx] = 0.0
    structural = np.broadcast_to(structure_params, (B, N, N)).astype(np.float32).copy()
    return adjacency, structural
